# revision 43
# baseline (speedup 1.0000x reference)
"""Multi-head causal attention on 8 TRN2 NeuronCores.

Problem: B=4, S=2048, D=768, H=12 heads (dk=64), causal mask, f32.

Sharding: 8 cores = 4 batches x 2 head-groups (6 heads each).
Core c handles batch c//2 and heads [6*(c%2), 6*(c%2)+6).
Each core computes its partial output projection (over its 384 local
features); the pair-sum and the bo bias add happen at unshard time on
the host.

Per-core kernel layout:
  xt  [768,2048]  = x[b].T          (bf16)
  wq/wk/wv [768,384] = W[hslice].T
  wo  [384,768]   = Wo[:, fslice].T
  v  = xt.T @ wv   [2048,384] natural layout, stored per (parity,pair)
       with a ones column per head for the softmax denominator:
       even head: [v(64) | 1 | 0...], odd head: [1 | 0... | v(64)]
       so pctx rows are: even -> l at row 64, odd -> l at row 0 and
       ctx rows land at 0:64 / 64:128 matching ctxt's feature layout.
  qT/kT = w.T @ xt [384,2048]  (dk-major rows)
Attention, flash-style per (pair mh, 512-wide i-block):
  scoresT pair-packed in one PSUM tile [128 j, 2 heads, 512 i];
  one ScalarE exp over both heads; causal mask applied POST-exp on
  the diagonal 128-tile via DVE multiply with a 0/1 triangle;
  ctx accumulated per head into [128,512] PSUM (1 bank each);
  epilogue: l rows (64/0) copied to SBUF, broadcast to all 128
  partitions with a single selector matmul, reciprocal on DVE,
  two partition-aligned DVE multiplies into ctxt.
Scheduling (per-engine queues execute in emission order, so emission
IS the schedule):
  - scores run LAG=2 j-tiles ahead of ctx (double-buffered PSUM);
  - the previous unit's epilogue is split: DVE-only part at unit
    start, PE broadcast + multiplies after the first scores;
  - within the attention phase ScalarE paces at ~1.0us/j-tile while
    the unit's own scores+ctx need only ~0.75us/j-tile of PE, so
    every unit carries filler PE work (remaining projection segments,
    deferred v-projection s-tiles, output projections) sized to its
    slack; units are ordered ib0/ib1 pairs first, then ib2, then ib3
    so qk/v projections for later pairs/tiles defer as late as their
    dependencies allow and output projection fills the final units;
  - all output is stored bf16 (host accumulates the core pairs in
    f32), halving the terminal DMA drain.
PSUM budget (8 banks): scores 2x[128,2,512] (4) + pctx 2x[128,512]
(2) + shared proj/outproj/broadcast pool (2).
"""

import os
import numpy as np
import ml_dtypes

import concourse.bass as bass
import concourse.tile as tile
import concourse.mybir as mybir
from concourse import bacc

B, S, D, H = 4, 2048, 768, 12
DK, P = 64, 128
HL = H // 2            # 6 heads per core
DL = HL * DK           # 384 local features
KD = D // P            # 6 contraction chunks over d
MT = DL // P           # 3 row-tiles of qT/kT/ctxT (= head pairs)
ST = S // P            # 16 s-tiles
IB = 512               # i-block width (1 PSUM bank of f32 per head)
NIB = S // IB          # 4 i-blocks

CDT = mybir.dt.bfloat16
NP_CDT = ml_dtypes.bfloat16
F32 = mybir.dt.float32

N_CORES = 8


def _emit(nc, tc, xt_d, wq_d, wk_d, wv_d, wo_d, out_d):
    Exp = mybir.ActivationFunctionType.Exp

    with tc.tile_pool(name="persist", bufs=1) as per, \
         tc.tile_pool(name="ps", bufs=2, space="PSUM") as pp, \
         tc.tile_pool(name="pc", bufs=2, space="PSUM") as cp, \
         tc.tile_pool(name="po", bufs=2, space="PSUM") as op, \
         tc.tile_pool(name="sb_e", bufs=6) as ebp, \
         tc.tile_pool(name="sb_r", bufs=2) as rbp, \
         tc.tile_pool(name="sb_o", bufs=4) as ob:
        xt = per.tile([P, KD, S], CDT)
        wq = per.tile([P, KD, DL], CDT)
        wk = per.tile([P, KD, DL], CDT)
        wv = per.tile([P, KD, DL], CDT)
        wo = per.tile([P, MT, D], CDT)
        qt = per.tile([P, MT, S], CDT)
        kt = per.tile([P, MT, S], CDT)
        # v2[j, st, parity, pair, e]: even head -> v at e 0:64, ones at 64
        #                             odd head  -> ones at 0, v at e 64:128
        v2 = per.tile([P, ST, 2, MT, P], CDT)
        ctxt = per.tile([P, MT, S], CDT)
        tri = per.tile([P, 2, P], CDT)    # 0/1 lower triangle, x2 heads
        sel = per.tile([P, P], CDT)       # denominator broadcast selector
        ls = [per.tile([P, IB], CDT, name=f"ls{i}") for i in range(2)]

        # --- input DMAs: v-projection path first, wo last ---
        # DRAM keeps the (k p) partition-interleaved layout (consecutive
        # rows hit consecutive SBUF partitions — p-major order serializes
        # on a single partition's write port and halves DMA bandwidth).
        # Transfers are serial per queue, so order = arrival order:
        # wv + xt[0:512] unblock vproj(0..3), then wk/wq for the pair-0
        # q/k projection, the rest streams behind.
        # a single DMA queue moves only ~100GB/s, so spread the inputs
        # over four engine queues that all transfer concurrently:
        # weights on gpsimd, the xt slices k-split across sync/vector/
        # scalar.  The first 512 i-columns (everything vproj(0..3) and
        # the pair-0 q/k projection need) land in ~3us instead of ~8.
        xtr = xt_d.rearrange("(k p) s -> p k s", p=P)
        nc.gpsimd.dma_start(out=wv, in_=wv_d.rearrange("(k p) e -> p k e", p=P))
        nc.gpsimd.dma_start(out=wk, in_=wk_d.rearrange("(k p) e -> p k e", p=P))
        nc.gpsimd.dma_start(out=wq, in_=wq_d.rearrange("(k p) e -> p k e", p=P))
        nc.gpsimd.dma_start(out=wo, in_=wo_d.rearrange("(m p) e -> p m e", p=P))
        nc.sync.dma_start(out=xt[:, :, 0:IB], in_=xtr[:, :, 0:IB])
        nc.sync.dma_start(out=xt[:, :, IB:2 * IB], in_=xtr[:, :, IB:2 * IB])
        nc.sync.dma_start(out=xt[:, :, 2 * IB:S], in_=xtr[:, :, 2 * IB:S])

        # --- constants ---
        nc.vector.memset(sel, 0.0)
        nc.vector.memset(sel[0:1, DK:P], 1.0)      # row 0 (l of odd head)
        nc.vector.memset(sel[DK:DK + 1, 0:DK], 1.0)  # row 64 (l of even head)
        for l in ls:
            nc.vector.memset(l, 0.0)
        nc.vector.memset(tri, 1.0)
        for b2 in range(2):
            nc.gpsimd.affine_select(
                out=tri[:, b2, :], in_=tri[:, b2, :],
                compare_op=mybir.AluOpType.is_ge,
                fill=0.0, base=0, pattern=[[1, P]], channel_multiplier=-1)
        # v2 needs only its ones columns initialized: the zero bands
        # (cols 65:128 of parity 0, 1:64 of parity 1) feed ctx PSUM rows
        # that are never read, so whatever junk SBUF holds there is
        # harmless — and skipping the big zero-fill keeps the early DVE
        # queue free for the first projection copies
        nc.vector.memset(v2[:, :, 0, :, DK], 1.0)
        nc.vector.memset(v2[:, :, 1, :, 0], 1.0)

        # HAM warmup: keep the PE active while input DMAs land so the
        # clock gate is released (2.4 GHz) when real work starts.
        wup = op.tile([P, IB], F32, tag="po", name="warmup")
        for _ in range(36):
            nc.tensor.matmul(wup[:, 0:P], lhsT=sel, rhs=sel,
                             start=True, stop=True)

        # --- phase 1 emitters (also used as fillers inside attention) ---
        # v projection for one s-tile: natural [s, e] layout, per-head slots
        def emit_vproj_st(st):
            ps = op.tile([P, MT, P], F32, tag="po", name=f"pv_{st}")
            for k in range(KD):
                nc.tensor.matmul(
                    ps[:, :, :], lhsT=xt[:, k, st * P:(st + 1) * P],
                    rhs=wv[:, k, :], start=(k == 0), stop=(k == KD - 1))
            nc.vector.tensor_copy(v2[:, st, 0, :, 0:DK], ps[:, :, 0:DK])
            nc.vector.tensor_copy(v2[:, st, 1, :, DK:P], ps[:, :, DK:P])

        # one 512-wide q/k projection segment (which: 0 = k, 1 = q)
        def emit_qkseg(mh, which, sg):
            wt, dst = ((wk, kt), (wq, qt))[which]
            ps = op.tile([P, IB], F32, tag="po", name=f"pqk_{mh}_{which}_{sg}")
            for k in range(KD):
                nc.tensor.matmul(
                    ps, lhsT=wt[:, k, mh * P:(mh + 1) * P],
                    rhs=xt[:, k, sg * IB:(sg + 1) * IB],
                    start=(k == 0), stop=(k == KD - 1))
            nc.vector.tensor_copy(dst[:, mh, sg * IB:(sg + 1) * IB], ps)

        # --- phase 2: attention units in one global software pipeline ---
        # Units (head-pair mh, i-block ib) share a single global scores
        # stream; ctx matmuls trail LAG j-tiles behind ACROSS unit
        # boundaries so the PE queue never drains at a unit switch.
        class Unit:
            def __init__(self, mh, ib, fillers):
                self.mh, self.ib = mh, ib
                self.i0 = ib * IB
                self.njt = (self.i0 + IB) // P
                self.fillers = list(fillers)
                self.pA = cp.tile([P, IB], F32, tag="pc", name=f"pA_{mh}_{ib}")
                self.pB = cp.tile([P, IB], F32, tag="pc", name=f"pB_{mh}_{ib}")
                self.ets = {}

            def scores(self, jt):
                mh, i0 = self.mh, self.i0
                c0 = max(0, jt * P - i0)
                ps = pp.tile([P, 2, IB], F32, tag="ps",
                             name=f"psc_{mh}_{self.ib}_{jt}")
                for h01 in range(2):
                    oh = h01 * DK
                    nc.tensor.matmul(
                        ps[:, h01, c0:IB],
                        lhsT=kt[oh:oh + DK, mh, jt * P:(jt + 1) * P],
                        rhs=qt[oh:oh + DK, mh, i0 + c0:i0 + IB],
                        start=True, stop=True)
                et = ebp.tile([P, 2, IB], CDT, tag="et")
                nc.scalar.activation(et[:, :, c0:IB], ps[:, :, c0:IB],
                                     Exp, scale=0.125)
                if jt * P >= i0:  # diagonal tile: zero keys above diag
                    # on GpSimd: its queue is empty, so the mask starts
                    # the moment the exp lands instead of waiting behind
                    # DVE casts — ctx of diagonal tiles unblocks sooner
                    nc.gpsimd.tensor_mul(et[:, :, c0:c0 + P],
                                         et[:, :, c0:c0 + P], tri)
                self.ets[jt] = (et, c0)

            def ctx(self, jt):
                et, c0 = self.ets.pop(jt)
                for h01, px in ((0, self.pA), (1, self.pB)):
                    nc.tensor.matmul(
                        px[:, c0:IB], lhsT=v2[:, jt, h01, self.mh, :],
                        rhs=et[:, h01, c0:IB],
                        start=(jt == 0), stop=(jt == self.njt - 1))

            def epilogue_early(self):
                # l rows: even head at pA row 64, odd head at pB row 0
                l = ls[(self.ib * MT + self.mh) % 2]
                nc.vector.tensor_copy(l[DK:DK + 1, :], self.pA[DK:DK + 1, :])
                nc.vector.tensor_copy(l[0:1, :], self.pB[0:1, :])

            def epilogue_late(self):
                mh, i0 = self.mh, self.i0
                l = ls[(self.ib * MT + mh) % 2]
                bl = op.tile([P, IB], F32, tag="po", name=f"bl_{mh}_{self.ib}")
                nc.tensor.matmul(bl, lhsT=sel, rhs=l, start=True, stop=True)
                rb = rbp.tile([P, IB], F32, tag="rb")
                nc.vector.reciprocal_approx_fast(rb, bl)
                nc.vector.tensor_mul(ctxt[0:DK, mh, i0:i0 + IB],
                                     self.pA[0:DK, :], rb[0:DK, :])
                nc.vector.tensor_mul(ctxt[DK:P, mh, i0:i0 + IB],
                                     self.pB[DK:P, :], rb[DK:P, :])

        # --- phase 3: output projection for one s-tile (bf16 store) ---
        def emit_outproj_st(st):
            po1 = op.tile([P, IB], F32, tag="po", name=f"po1_{st}")
            po2 = op.tile([P, D - IB], F32, tag="po", name=f"po2_{st}")
            # groups interleaved m-major so the last pair's chunk (which
            # may wait on a just-flushed epilogue) gates only the tail
            for m in range(MT):
                for (pt, n0, nn) in ((po1, 0, IB), (po2, IB, D - IB)):
                    nc.tensor.matmul(
                        pt[:, 0:nn],
                        lhsT=ctxt[:, m, st * P:(st + 1) * P],
                        rhs=wo[:, m, n0:n0 + nn],
                        start=(m == 0), stop=(m == MT - 1))
            osb = ob.tile([P, D], CDT, tag="osb")
            nc.vector.tensor_copy(osb[:, 0:IB], po1)
            nc.vector.tensor_copy(osb[:, IB:D], po2)
            nc.sync.dma_start(out=out_d[st * P:(st + 1) * P, :], in_=osb)

        # filler chunks: half a q/k segment (3 matmuls) or half a
        # v-projection s-tile keeps the per-step PE granule close to the
        # per-j-tile ScalarE pacing deficit; outproj stays whole (its
        # two PSUM tiles must not straddle another po-pool allocation).
        # filler chunks of ~2 matmuls (~0.45us) match the per-j-tile
        # ScalarE pacing deficit, so the PE queue drains smoothly.  A
        # chunk sequence holds one po-pool tile open across up to 3
        # steps; the driver defers epilogue_late (which also allocates
        # from the po pool) while a sequence is open — interleaving the
        # two would emit an allocation whose WAR release sits later in
        # the same PE queue (deadlock).
        _qk_ps = {}

        def Fc(mh, which, sg, k0, on_scalar=False):
            wt, dst = ((wk, kt), (wq, qt))[which]

            def go():
                if k0 == 0:
                    _qk_ps[(mh, which, sg)] = op.tile(
                        [P, IB], F32, tag="po", name=f"pqk_{mh}_{which}_{sg}")
                ps = _qk_ps[(mh, which, sg)]
                for k in range(k0, k0 + 2):
                    nc.tensor.matmul(
                        ps, lhsT=wt[:, k, mh * P:(mh + 1) * P],
                        rhs=xt[:, k, sg * IB:(sg + 1) * IB],
                        start=(k == 0), stop=(k == KD - 1))
                if k0 == 4:
                    # PE-bound stretches put the PSUM->SBUF cast on the
                    # (idle) ScalarE so the po pool recycles without
                    # waiting on the DVE queue
                    if on_scalar:
                        nc.scalar.copy(dst[:, mh, sg * IB:(sg + 1) * IB], ps)
                    else:
                        nc.vector.tensor_copy(
                            dst[:, mh, sg * IB:(sg + 1) * IB], ps)
            return go

        def F2(mh, which, sg, on_scalar=False):
            return [(Fc(mh, which, sg, 0, on_scalar), True),
                    (Fc(mh, which, sg, 2, on_scalar), True),
                    (Fc(mh, which, sg, 4, on_scalar), False)]

        _v_ps = {}

        def Vc(st, k0, on_scalar=False):
            def go():
                if k0 == 0:
                    _v_ps[st] = op.tile([P, MT, P], F32, tag="po",
                                        name=f"pv_{st}")
                ps = _v_ps[st]
                for k in range(k0, k0 + 2):
                    nc.tensor.matmul(
                        ps[:, :, :], lhsT=xt[:, k, st * P:(st + 1) * P],
                        rhs=wv[:, k, :], start=(k == 0), stop=(k == KD - 1))
                if k0 == 4:
                    if on_scalar:
                        nc.scalar.copy(v2[:, st, 0, :, 0:DK], ps[:, :, 0:DK])
                        nc.scalar.copy(v2[:, st, 1, :, DK:P], ps[:, :, DK:P])
                    else:
                        nc.vector.tensor_copy(v2[:, st, 0, :, 0:DK],
                                              ps[:, :, 0:DK])
                        nc.vector.tensor_copy(v2[:, st, 1, :, DK:P],
                                              ps[:, :, DK:P])
            return go

        def V2(st, on_scalar=False):
            return [(Vc(st, 0, on_scalar), True), (Vc(st, 2, on_scalar), True),
                    (Vc(st, 4, on_scalar), False)]

        # outproj as a filler splits in two ~0.7us chunks (the po1/po2
        # tiles allocate in consecutive steps; anything >~2us of
        # non-scores PE work in a row starves the 2-deep ACT lookahead)
        _o_ps = {}

        def Oc(st, half):
            def go():
                if half == 0:
                    _o_ps[st] = op.tile([P, IB], F32, tag="po",
                                        name=f"po1_{st}")
                    for m in range(MT):
                        nc.tensor.matmul(
                            _o_ps[st][:, 0:IB],
                            lhsT=ctxt[:, m, st * P:(st + 1) * P],
                            rhs=wo[:, m, 0:IB],
                            start=(m == 0), stop=(m == MT - 1))
                else:
                    po2 = op.tile([P, D - IB], F32, tag="po",
                                  name=f"po2_{st}")
                    for m in range(MT):
                        nc.tensor.matmul(
                            po2[:, 0:D - IB],
                            lhsT=ctxt[:, m, st * P:(st + 1) * P],
                            rhs=wo[:, m, IB:D],
                            start=(m == 0), stop=(m == MT - 1))
                    osb = ob.tile([P, D], CDT, tag="osb")
                    nc.vector.tensor_copy(osb[:, 0:IB], _o_ps[st])
                    nc.vector.tensor_copy(osb[:, IB:D], po2)
                    nc.sync.dma_start(out=out_d[st * P:(st + 1) * P, :],
                                      in_=osb)
            return go

        def O(st):
            return [(Oc(st, 0), True), (Oc(st, 1), False)]

        units = [
            # k-segment projections ride at the HEAD of the unit that
            # first consumes them (seg N's first consumer is j-tile 4N =
            # in-unit step 4N, and head chunks finish by step ~2), which
            # shifts PE filler work out of the over-stuffed early units
            # into the chunk-starved late ones
            Unit(0, 0, F2(0, 1, 1) + V2(4) + V2(5)),
            Unit(0, 1, F2(0, 0, 1) + F2(1, 0, 0) + F2(1, 1, 0)
                 + V2(6) + V2(7) + F2(1, 1, 1)),
            Unit(1, 0, F2(2, 0, 0) + F2(2, 1, 0) + V2(8) + V2(9)),
            Unit(1, 1, F2(1, 0, 1) + F2(2, 1, 1) + V2(10) + V2(11)
                 + F2(0, 1, 2)),
            Unit(2, 0, F2(1, 1, 2)),
            Unit(2, 1, F2(2, 0, 1) + F2(0, 1, 3) + F2(2, 1, 2)),
            Unit(0, 2, F2(0, 0, 2) + F2(1, 1, 3) + V2(12) + V2(13)),
            Unit(1, 2, F2(1, 0, 2) + F2(2, 1, 3) + V2(14) + V2(15)),
            Unit(2, 2, F2(2, 0, 2) + O(0) + O(1) + O(2) + O(3)),
            Unit(0, 3, F2(0, 0, 3) + O(4) + O(5) + O(6) + O(7)),
            Unit(1, 3, F2(1, 0, 3) + O(8) + O(9) + O(10)),
            Unit(2, 3, F2(2, 0, 3) + O(11)),
        ]

        # preamble: just enough projection for unit (0,0) to start;
        # vproj(0..3) only needs the first xt slice, so it runs while
        # wk/wq are still in flight
        for st in range(4):
            emit_vproj_st(st)
        emit_qkseg(0, 0, 0)
        emit_qkseg(0, 1, 0)

        # global pipeline driver: one scores step per iteration; ctx
        # trails LAG steps behind across unit boundaries; epilogues
        # emit as soon as their unit's last ctx lands; one filler chunk
        # per step from the current unit's queue (leftovers spill at
        # the unit boundary).
        LAG = 2
        from collections import deque
        ctxq = deque()
        late_epi = None

        def drain_one():
            nonlocal late_epi
            u, cjt = ctxq.popleft()
            u.ctx(cjt)
            if cjt == u.njt - 1:
                assert late_epi is None, "previous epilogue_late never emitted"
                u.epilogue_early()
                late_epi = u

        # per-step emission order matters: the PE queue executes in
        # order, so the ACT-gated instructions (ctx of jt-LAG, then this
        # step's scores) go LAST and the independent work (epilogue
        # broadcast, filler chunk) goes first to absorb the ACT pacing
        # deficit without blocking the queue head.
        filler_open = False
        for u in units:
            for jt in range(u.njt):
                if late_epi is not None and not filler_open:
                    late_epi.epilogue_late()
                    late_epi = None
                if u.fillers:
                    fn, filler_open = u.fillers.pop(0)
                    fn()
                if len(ctxq) >= LAG:
                    drain_one()
                u.scores(jt)
                ctxq.append((u, jt))
            for fn, _ in u.fillers:
                fn()
            filler_open = False
            u.fillers = []
        while ctxq:
            drain_one()
        if late_epi is not None:
            late_epi.epilogue_late()
            late_epi = None
        # tail: the last i-block's output projection.  The scores/pctx
        # pools are idle now, so each s-tile gets its own PSUM space
        # (po1+po2 packed into one [128,2,512] scores-shaped tile or a
        # pctx tile pair) — no pool WAR serialization across the four
        # tiles, they pipeline at PE/DVE/DMA throughput.
        def emit_outproj_tail(st, po1, po2, dmaq):
            for m in range(MT):
                nc.tensor.matmul(
                    po1, lhsT=ctxt[:, m, st * P:(st + 1) * P],
                    rhs=wo[:, m, 0:IB], start=(m == 0), stop=(m == MT - 1))
            for m in range(MT):
                nc.tensor.matmul(
                    po2, lhsT=ctxt[:, m, st * P:(st + 1) * P],
                    rhs=wo[:, m, IB:D], start=(m == 0), stop=(m == MT - 1))
            # split the cast so the first half's DMA starts while the
            # second half converts; the four tiles' stores go out on
            # different DMA queues and drain in parallel
            osb = ob.tile([P, D], CDT, tag="osb")
            nc.vector.tensor_copy(osb[:, 0:IB], po1)
            dmaq.dma_start(out=out_d[st * P:(st + 1) * P, 0:IB],
                           in_=osb[:, 0:IB])
            nc.vector.tensor_copy(osb[:, IB:D], po2)
            dmaq.dma_start(out=out_d[st * P:(st + 1) * P, IB:D],
                           in_=osb[:, IB:D])

        tp1 = pp.tile([P, 2, IB], F32, tag="ps", name="tail12")
        emit_outproj_tail(12, tp1[:, 0, :], tp1[:, 1, 0:D - IB], nc.sync)
        tp2 = pp.tile([P, 2, IB], F32, tag="ps", name="tail13")
        emit_outproj_tail(13, tp2[:, 0, :], tp2[:, 1, 0:D - IB], nc.scalar)
        tc1 = cp.tile([P, IB], F32, tag="pc", name="tail14a")
        tc2 = cp.tile([P, IB], F32, tag="pc", name="tail14b")
        emit_outproj_tail(14, tc1, tc2[:, 0:D - IB], nc.gpsimd)
        to1 = op.tile([P, IB], F32, tag="po", name="tail15a")
        to2 = op.tile([P, D - IB], F32, tag="po", name="tail15b")
        emit_outproj_tail(15, to1, to2, nc.sync)


def build_nc():
    nc = bacc.Bacc(trn_type="TRN2", target_bir_lowering=False, debug=False)
    xt_d = nc.dram_tensor("xt", [D, S], CDT, kind="ExternalInput").ap()
    wq_d = nc.dram_tensor("wq", [D, DL], CDT, kind="ExternalInput").ap()
    wk_d = nc.dram_tensor("wk", [D, DL], CDT, kind="ExternalInput").ap()
    wv_d = nc.dram_tensor("wv", [D, DL], CDT, kind="ExternalInput").ap()
    wo_d = nc.dram_tensor("wo", [DL, D], CDT, kind="ExternalInput").ap()
    out_d = nc.dram_tensor("out", [S, D], CDT, kind="ExternalOutput").ap()
    with tile.TileContext(nc) as tc:
        _emit(nc, tc, xt_d, wq_d, wk_d, wv_d, wo_d, out_d)
    nc.compile()
    return nc


def make_in_maps(x, Wq, Wk, Wv, Wo):
    in_maps = []
    for c in range(N_CORES):
        b, g = c // 2, c % 2
        hsl = slice(g * DL, (g + 1) * DL)
        in_maps.append({
            "xt": np.ascontiguousarray(x[b].T).astype(NP_CDT),
            "wq": np.ascontiguousarray(Wq[hsl, :].T).astype(NP_CDT),
            "wk": np.ascontiguousarray(Wk[hsl, :].T).astype(NP_CDT),
            "wv": np.ascontiguousarray(Wv[hsl, :].T).astype(NP_CDT),
            "wo": np.ascontiguousarray(Wo[:, hsl].T).astype(NP_CDT),
        })
    return in_maps


_BUILT = None
LAST_RESULT = None


def _install_ntff_hook():
    """Provide the antenv.axon_hooks module run_bass_kernel_spmd expects
    for NTFF profiling under axon (the agent image ships only a stub
    antenv package)."""
    import sys
    import types
    if "antenv.axon_hooks" in sys.modules:
        return
    mod = types.ModuleType("antenv.axon_hooks")
    mod._hook = None

    def set_axon_ntff_profile_hook(h):
        mod._hook = h

    def get_axon_ntff_profile_hook():
        return mod._hook

    mod.set_axon_ntff_profile_hook = set_axon_ntff_profile_hook
    mod.get_axon_ntff_profile_hook = get_axon_ntff_profile_hook
    sys.modules["antenv.axon_hooks"] = mod
    import antenv
    antenv.axon_hooks = mod
    try:
        from trn_agent_boot.trn_boot import _ntff_profile_via_ctypes
        hook = _ntff_profile_via_ctypes("/opt/axon/libaxon_pjrt.so")
        if hook is not None:
            mod._hook = hook
    except Exception:
        pass


def kernel(**inputs):
    global _BUILT, LAST_RESULT
    from concourse.bass_utils import run_bass_kernel_spmd

    x = np.asarray(inputs["x"], np.float32)
    Wq = np.asarray(inputs["Wq"], np.float32)
    Wk = np.asarray(inputs["Wk"], np.float32)
    Wv = np.asarray(inputs["Wv"], np.float32)
    Wo = np.asarray(inputs["Wo"], np.float32)
    bo = np.asarray(inputs["bo"], np.float32)

    if _BUILT is None:
        _BUILT = build_nc()
    nc = _BUILT

    trace = bool(int(os.environ.get("KTRACE", "0")))
    if trace:
        _install_ntff_hook()
    in_maps = make_in_maps(x, Wq, Wk, Wv, Wo)
    res = run_bass_kernel_spmd(
        nc, in_maps, core_ids=list(range(N_CORES)), trace=trace)
    LAST_RESULT = res

    out = np.empty((B, S, D), np.float32)
    for b in range(B):
        out[b] = np.asarray(res.results[2 * b]["out"], np.float32)
        out[b] += np.asarray(res.results[2 * b + 1]["out"], np.float32)
    out += bo
    return out


# revision 44
# speedup vs baseline: 1.0068x; 1.0068x over previous
"""Multi-head causal attention on 8 TRN2 NeuronCores.

Problem: B=4, S=2048, D=768, H=12 heads (dk=64), causal mask, f32.

Sharding: 8 cores = 4 batches x 2 head-groups (6 heads each).
Core c handles batch c//2 and heads [6*(c%2), 6*(c%2)+6).
Each core computes its partial output projection (over its 384 local
features); the pair-sum and the bo bias add happen at unshard time on
the host.

Per-core kernel layout:
  xt  [768,2048]  = x[b].T          (bf16)
  wq/wk/wv [768,384] = W[hslice].T
  wo  [384,768]   = Wo[:, fslice].T
  v  = xt.T @ wv   [2048,384] natural layout, stored per (parity,pair)
       with a ones column per head for the softmax denominator:
       even head: [v(64) | 1 | 0...], odd head: [1 | 0... | v(64)]
       so pctx rows are: even -> l at row 64, odd -> l at row 0 and
       ctx rows land at 0:64 / 64:128 matching ctxt's feature layout.
  qT/kT = w.T @ xt [384,2048]  (dk-major rows)
Attention, flash-style per (pair mh, 512-wide i-block):
  scoresT pair-packed in one PSUM tile [128 j, 2 heads, 512 i];
  one ScalarE exp over both heads; causal mask applied POST-exp on
  the diagonal 128-tile via DVE multiply with a 0/1 triangle;
  ctx accumulated per head into [128,512] PSUM (1 bank each);
  epilogue: l rows (64/0) copied to SBUF, broadcast to all 128
  partitions with a single selector matmul, reciprocal on DVE,
  two partition-aligned DVE multiplies into ctxt.
Scheduling (per-engine queues execute in emission order, so emission
IS the schedule):
  - scores run LAG=2 j-tiles ahead of ctx (double-buffered PSUM);
  - the previous unit's epilogue is split: DVE-only part at unit
    start, PE broadcast + multiplies after the first scores;
  - within the attention phase ScalarE paces at ~1.0us/j-tile while
    the unit's own scores+ctx need only ~0.75us/j-tile of PE, so
    every unit carries filler PE work (remaining projection segments,
    deferred v-projection s-tiles, output projections) sized to its
    slack; units are ordered ib0/ib1 pairs first, then ib2, then ib3
    so qk/v projections for later pairs/tiles defer as late as their
    dependencies allow and output projection fills the final units;
  - all output is stored bf16 (host accumulates the core pairs in
    f32), halving the terminal DMA drain.
PSUM budget (8 banks): scores 2x[128,2,512] (4) + pctx 2x[128,512]
(2) + shared proj/outproj/broadcast pool (2).
"""

import os
import numpy as np
import ml_dtypes

import concourse.bass as bass
import concourse.tile as tile
import concourse.mybir as mybir
from concourse import bacc

B, S, D, H = 4, 2048, 768, 12
DK, P = 64, 128
HL = H // 2            # 6 heads per core
DL = HL * DK           # 384 local features
KD = D // P            # 6 contraction chunks over d
MT = DL // P           # 3 row-tiles of qT/kT/ctxT (= head pairs)
ST = S // P            # 16 s-tiles
IB = 512               # i-block width (1 PSUM bank of f32 per head)
NIB = S // IB          # 4 i-blocks

CDT = mybir.dt.bfloat16
NP_CDT = ml_dtypes.bfloat16
F32 = mybir.dt.float32

N_CORES = 8


def _emit(nc, tc, xt_d, wq_d, wk_d, wv_d, wo_d, out_d):
    Exp = mybir.ActivationFunctionType.Exp

    with tc.tile_pool(name="persist", bufs=1) as per, \
         tc.tile_pool(name="ps", bufs=2, space="PSUM") as pp, \
         tc.tile_pool(name="pc", bufs=2, space="PSUM") as cp, \
         tc.tile_pool(name="po", bufs=2, space="PSUM") as op, \
         tc.tile_pool(name="sb_e", bufs=6) as ebp, \
         tc.tile_pool(name="sb_r", bufs=2) as rbp, \
         tc.tile_pool(name="sb_o", bufs=4) as ob:
        xt = per.tile([P, KD, S], CDT)
        wq = per.tile([P, KD, DL], CDT)
        wk = per.tile([P, KD, DL], CDT)
        wv = per.tile([P, KD, DL], CDT)
        wo = per.tile([P, MT, D], CDT)
        qt = per.tile([P, MT, S], CDT)
        kt = per.tile([P, MT, S], CDT)
        # v2[j, st, parity, pair, e]: even head -> v at e 0:64, ones at 64
        #                             odd head  -> ones at 0, v at e 64:128
        v2 = per.tile([P, ST, 2, MT, P], CDT)
        ctxt = per.tile([P, MT, S], CDT)
        tri = per.tile([P, 2, P], CDT)    # 0/1 lower triangle, x2 heads
        sel = per.tile([P, P], CDT)       # denominator broadcast selector
        ls = [per.tile([P, IB], CDT, name=f"ls{i}") for i in range(2)]

        # --- input DMAs: v-projection path first, wo last ---
        # DRAM keeps the (k p) partition-interleaved layout (consecutive
        # rows hit consecutive SBUF partitions — p-major order serializes
        # on a single partition's write port and halves DMA bandwidth).
        # Transfers are serial per queue, so order = arrival order:
        # wv + xt[0:512] unblock vproj(0..3), then wk/wq for the pair-0
        # q/k projection, the rest streams behind.
        # a single DMA queue moves only ~100GB/s, so spread the inputs
        # over four engine queues that all transfer concurrently:
        # weights on gpsimd, the xt slices k-split across sync/vector/
        # scalar.  The first 512 i-columns (everything vproj(0..3) and
        # the pair-0 q/k projection need) land in ~3us instead of ~8.
        xtr = xt_d.rearrange("(k p) s -> p k s", p=P)
        nc.gpsimd.dma_start(out=wv, in_=wv_d.rearrange("(k p) e -> p k e", p=P))
        nc.gpsimd.dma_start(out=wk, in_=wk_d.rearrange("(k p) e -> p k e", p=P))
        nc.gpsimd.dma_start(out=wq, in_=wq_d.rearrange("(k p) e -> p k e", p=P))
        nc.gpsimd.dma_start(out=wo, in_=wo_d.rearrange("(m p) e -> p m e", p=P))
        nc.sync.dma_start(out=xt[:, :, 0:IB], in_=xtr[:, :, 0:IB])
        nc.sync.dma_start(out=xt[:, :, IB:2 * IB], in_=xtr[:, :, IB:2 * IB])
        nc.sync.dma_start(out=xt[:, :, 2 * IB:S], in_=xtr[:, :, 2 * IB:S])

        # --- constants ---
        nc.vector.memset(sel, 0.0)
        nc.vector.memset(sel[0:1, DK:P], 1.0)      # row 0 (l of odd head)
        nc.vector.memset(sel[DK:DK + 1, 0:DK], 1.0)  # row 64 (l of even head)
        for l in ls:
            nc.vector.memset(l, 0.0)
        nc.vector.memset(tri, 1.0)
        for b2 in range(2):
            nc.gpsimd.affine_select(
                out=tri[:, b2, :], in_=tri[:, b2, :],
                compare_op=mybir.AluOpType.is_ge,
                fill=0.0, base=0, pattern=[[1, P]], channel_multiplier=-1)
        # v2 needs only its ones columns initialized: the zero bands
        # (cols 65:128 of parity 0, 1:64 of parity 1) feed ctx PSUM rows
        # that are never read, so whatever junk SBUF holds there is
        # harmless — and skipping the big zero-fill keeps the early DVE
        # queue free for the first projection copies
        nc.vector.memset(v2[:, :, 0, :, DK], 1.0)
        nc.vector.memset(v2[:, :, 1, :, 0], 1.0)

        # HAM warmup: keep the PE active while input DMAs land so the
        # clock gate is released (2.4 GHz) when real work starts.
        wup = op.tile([P, IB], F32, tag="po", name="warmup")
        for _ in range(36):
            nc.tensor.matmul(wup[:, 0:P], lhsT=sel, rhs=sel,
                             start=True, stop=True)

        # --- phase 1 emitters (also used as fillers inside attention) ---
        # v projection for one s-tile: natural [s, e] layout, per-head slots
        def emit_vproj_st(st):
            ps = op.tile([P, MT, P], F32, tag="po", name=f"pv_{st}")
            for k in range(KD):
                nc.tensor.matmul(
                    ps[:, :, :], lhsT=xt[:, k, st * P:(st + 1) * P],
                    rhs=wv[:, k, :], start=(k == 0), stop=(k == KD - 1))
            nc.vector.tensor_copy(v2[:, st, 0, :, 0:DK], ps[:, :, 0:DK])
            nc.vector.tensor_copy(v2[:, st, 1, :, DK:P], ps[:, :, DK:P])

        # one 512-wide q/k projection segment (which: 0 = k, 1 = q)
        def emit_qkseg(mh, which, sg):
            wt, dst = ((wk, kt), (wq, qt))[which]
            ps = op.tile([P, IB], F32, tag="po", name=f"pqk_{mh}_{which}_{sg}")
            for k in range(KD):
                nc.tensor.matmul(
                    ps, lhsT=wt[:, k, mh * P:(mh + 1) * P],
                    rhs=xt[:, k, sg * IB:(sg + 1) * IB],
                    start=(k == 0), stop=(k == KD - 1))
            nc.vector.tensor_copy(dst[:, mh, sg * IB:(sg + 1) * IB], ps)

        # --- phase 2: attention units in one global software pipeline ---
        # Units (head-pair mh, i-block ib) share a single global scores
        # stream; ctx matmuls trail LAG j-tiles behind ACROSS unit
        # boundaries so the PE queue never drains at a unit switch.
        class Unit:
            def __init__(self, mh, ib, fillers):
                self.mh, self.ib = mh, ib
                self.i0 = ib * IB
                self.njt = (self.i0 + IB) // P
                self.fillers = list(fillers)
                self.pA = cp.tile([P, IB], F32, tag="pc", name=f"pA_{mh}_{ib}")
                self.pB = cp.tile([P, IB], F32, tag="pc", name=f"pB_{mh}_{ib}")
                self.ets = {}

            def scores(self, jt):
                mh, i0 = self.mh, self.i0
                c0 = max(0, jt * P - i0)
                ps = pp.tile([P, 2, IB], F32, tag="ps",
                             name=f"psc_{mh}_{self.ib}_{jt}")
                for h01 in range(2):
                    oh = h01 * DK
                    nc.tensor.matmul(
                        ps[:, h01, c0:IB],
                        lhsT=kt[oh:oh + DK, mh, jt * P:(jt + 1) * P],
                        rhs=qt[oh:oh + DK, mh, i0 + c0:i0 + IB],
                        start=True, stop=True)
                et = ebp.tile([P, 2, IB], CDT, tag="et")
                nc.scalar.activation(et[:, :, c0:IB], ps[:, :, c0:IB],
                                     Exp, scale=0.125)
                if jt * P >= i0:  # diagonal tile: zero keys above diag
                    nc.vector.tensor_mul(et[:, :, c0:c0 + P],
                                         et[:, :, c0:c0 + P], tri)
                self.ets[jt] = (et, c0)

            def ctx(self, jt):
                et, c0 = self.ets.pop(jt)
                for h01, px in ((0, self.pA), (1, self.pB)):
                    nc.tensor.matmul(
                        px[:, c0:IB], lhsT=v2[:, jt, h01, self.mh, :],
                        rhs=et[:, h01, c0:IB],
                        start=(jt == 0), stop=(jt == self.njt - 1))

            def epilogue_early(self):
                # l rows: even head at pA row 64, odd head at pB row 0
                l = ls[(self.ib * MT + self.mh) % 2]
                nc.vector.tensor_copy(l[DK:DK + 1, :], self.pA[DK:DK + 1, :])
                nc.vector.tensor_copy(l[0:1, :], self.pB[0:1, :])

            def epilogue_late(self):
                mh, i0 = self.mh, self.i0
                l = ls[(self.ib * MT + mh) % 2]
                bl = op.tile([P, IB], F32, tag="po", name=f"bl_{mh}_{self.ib}")
                nc.tensor.matmul(bl, lhsT=sel, rhs=l, start=True, stop=True)
                rb = rbp.tile([P, IB], F32, tag="rb")
                nc.vector.reciprocal_approx_fast(rb, bl)
                nc.vector.tensor_mul(ctxt[0:DK, mh, i0:i0 + IB],
                                     self.pA[0:DK, :], rb[0:DK, :])
                nc.vector.tensor_mul(ctxt[DK:P, mh, i0:i0 + IB],
                                     self.pB[DK:P, :], rb[DK:P, :])

        # --- phase 3: output projection for one s-tile (bf16 store) ---
        def emit_outproj_st(st):
            po1 = op.tile([P, IB], F32, tag="po", name=f"po1_{st}")
            po2 = op.tile([P, D - IB], F32, tag="po", name=f"po2_{st}")
            # groups interleaved m-major so the last pair's chunk (which
            # may wait on a just-flushed epilogue) gates only the tail
            for m in range(MT):
                for (pt, n0, nn) in ((po1, 0, IB), (po2, IB, D - IB)):
                    nc.tensor.matmul(
                        pt[:, 0:nn],
                        lhsT=ctxt[:, m, st * P:(st + 1) * P],
                        rhs=wo[:, m, n0:n0 + nn],
                        start=(m == 0), stop=(m == MT - 1))
            osb = ob.tile([P, D], CDT, tag="osb")
            nc.vector.tensor_copy(osb[:, 0:IB], po1)
            nc.vector.tensor_copy(osb[:, IB:D], po2)
            nc.sync.dma_start(out=out_d[st * P:(st + 1) * P, :], in_=osb)

        # filler chunks: half a q/k segment (3 matmuls) or half a
        # v-projection s-tile keeps the per-step PE granule close to the
        # per-j-tile ScalarE pacing deficit; outproj stays whole (its
        # two PSUM tiles must not straddle another po-pool allocation).
        # filler chunks of ~2 matmuls (~0.45us) match the per-j-tile
        # ScalarE pacing deficit, so the PE queue drains smoothly.  A
        # chunk sequence holds one po-pool tile open across up to 3
        # steps; the driver defers epilogue_late (which also allocates
        # from the po pool) while a sequence is open — interleaving the
        # two would emit an allocation whose WAR release sits later in
        # the same PE queue (deadlock).
        _qk_ps = {}

        def Fc(mh, which, sg, k0, on_scalar=False):
            wt, dst = ((wk, kt), (wq, qt))[which]

            def go():
                if k0 == 0:
                    _qk_ps[(mh, which, sg)] = op.tile(
                        [P, IB], F32, tag="po", name=f"pqk_{mh}_{which}_{sg}")
                ps = _qk_ps[(mh, which, sg)]
                for k in range(k0, k0 + 2):
                    nc.tensor.matmul(
                        ps, lhsT=wt[:, k, mh * P:(mh + 1) * P],
                        rhs=xt[:, k, sg * IB:(sg + 1) * IB],
                        start=(k == 0), stop=(k == KD - 1))
                if k0 == 4:
                    # PE-bound stretches put the PSUM->SBUF cast on the
                    # (idle) ScalarE so the po pool recycles without
                    # waiting on the DVE queue
                    if on_scalar:
                        nc.scalar.copy(dst[:, mh, sg * IB:(sg + 1) * IB], ps)
                    else:
                        nc.vector.tensor_copy(
                            dst[:, mh, sg * IB:(sg + 1) * IB], ps)
            return go

        def F2(mh, which, sg, on_scalar=False):
            return [(Fc(mh, which, sg, 0, on_scalar), True),
                    (Fc(mh, which, sg, 2, on_scalar), True),
                    (Fc(mh, which, sg, 4, on_scalar), False)]

        _v_ps = {}

        def Vc(st, k0, on_scalar=False):
            def go():
                if k0 == 0:
                    _v_ps[st] = op.tile([P, MT, P], F32, tag="po",
                                        name=f"pv_{st}")
                ps = _v_ps[st]
                for k in range(k0, k0 + 2):
                    nc.tensor.matmul(
                        ps[:, :, :], lhsT=xt[:, k, st * P:(st + 1) * P],
                        rhs=wv[:, k, :], start=(k == 0), stop=(k == KD - 1))
                if k0 == 4:
                    if on_scalar:
                        nc.scalar.copy(v2[:, st, 0, :, 0:DK], ps[:, :, 0:DK])
                        nc.scalar.copy(v2[:, st, 1, :, DK:P], ps[:, :, DK:P])
                    else:
                        nc.vector.tensor_copy(v2[:, st, 0, :, 0:DK],
                                              ps[:, :, 0:DK])
                        nc.vector.tensor_copy(v2[:, st, 1, :, DK:P],
                                              ps[:, :, DK:P])
            return go

        def V2(st, on_scalar=False):
            return [(Vc(st, 0, on_scalar), True), (Vc(st, 2, on_scalar), True),
                    (Vc(st, 4, on_scalar), False)]

        # outproj as a filler splits in two ~0.7us chunks (the po1/po2
        # tiles allocate in consecutive steps; anything >~2us of
        # non-scores PE work in a row starves the 2-deep ACT lookahead)
        _o_ps = {}

        def Oc(st, half):
            def go():
                if half == 0:
                    _o_ps[st] = op.tile([P, IB], F32, tag="po",
                                        name=f"po1_{st}")
                    for m in range(MT):
                        nc.tensor.matmul(
                            _o_ps[st][:, 0:IB],
                            lhsT=ctxt[:, m, st * P:(st + 1) * P],
                            rhs=wo[:, m, 0:IB],
                            start=(m == 0), stop=(m == MT - 1))
                else:
                    po2 = op.tile([P, D - IB], F32, tag="po",
                                  name=f"po2_{st}")
                    for m in range(MT):
                        nc.tensor.matmul(
                            po2[:, 0:D - IB],
                            lhsT=ctxt[:, m, st * P:(st + 1) * P],
                            rhs=wo[:, m, IB:D],
                            start=(m == 0), stop=(m == MT - 1))
                    osb = ob.tile([P, D], CDT, tag="osb")
                    nc.vector.tensor_copy(osb[:, 0:IB], _o_ps[st])
                    nc.vector.tensor_copy(osb[:, IB:D], po2)
                    nc.sync.dma_start(out=out_d[st * P:(st + 1) * P, :],
                                      in_=osb)
            return go

        def O(st):
            return [(Oc(st, 0), True), (Oc(st, 1), False)]

        units = [
            # k-segment projections ride at the HEAD of the unit that
            # first consumes them (seg N's first consumer is j-tile 4N =
            # in-unit step 4N, and head chunks finish by step ~2), which
            # shifts PE filler work out of the over-stuffed early units
            # into the chunk-starved late ones
            Unit(0, 0, F2(0, 1, 1) + V2(4) + V2(5)),
            Unit(0, 1, F2(0, 0, 1) + F2(1, 0, 0) + F2(1, 1, 0)
                 + V2(6) + V2(7) + F2(1, 1, 1)),
            Unit(1, 0, F2(2, 0, 0) + F2(2, 1, 0) + V2(8) + V2(9)),
            Unit(1, 1, F2(1, 0, 1) + F2(2, 1, 1) + V2(10) + V2(11)
                 + F2(0, 1, 2)),
            Unit(2, 0, F2(1, 1, 2)),
            Unit(2, 1, F2(2, 0, 1) + F2(0, 1, 3) + F2(2, 1, 2)),
            Unit(0, 2, F2(0, 0, 2) + F2(1, 1, 3) + V2(12) + V2(13)),
            Unit(1, 2, F2(1, 0, 2) + F2(2, 1, 3) + V2(14) + V2(15)),
            Unit(2, 2, F2(2, 0, 2) + O(0) + O(1) + O(2) + O(3)),
            Unit(0, 3, F2(0, 0, 3) + O(4) + O(5) + O(6) + O(7)),
            Unit(1, 3, F2(1, 0, 3) + O(8) + O(9) + O(10)),
            Unit(2, 3, F2(2, 0, 3) + O(11)),
        ]

        # preamble: just enough projection for unit (0,0) to start;
        # vproj(0..3) only needs the first xt slice, so it runs while
        # wk/wq are still in flight
        for st in range(4):
            emit_vproj_st(st)
        emit_qkseg(0, 0, 0)
        emit_qkseg(0, 1, 0)

        # global pipeline driver: one scores step per iteration; ctx
        # trails LAG steps behind across unit boundaries; epilogues
        # emit as soon as their unit's last ctx lands; one filler chunk
        # per step from the current unit's queue (leftovers spill at
        # the unit boundary).
        LAG = 2
        from collections import deque
        ctxq = deque()
        late_epi = None

        def drain_one():
            nonlocal late_epi
            u, cjt = ctxq.popleft()
            u.ctx(cjt)
            if cjt == u.njt - 1:
                assert late_epi is None, "previous epilogue_late never emitted"
                u.epilogue_early()
                late_epi = u

        # per-step emission order matters: the PE queue executes in
        # order, so the ACT-gated instructions (ctx of jt-LAG, then this
        # step's scores) go LAST and the independent work (epilogue
        # broadcast, filler chunk) goes first to absorb the ACT pacing
        # deficit without blocking the queue head.
        filler_open = False
        for u in units:
            for jt in range(u.njt):
                if late_epi is not None and not filler_open:
                    late_epi.epilogue_late()
                    late_epi = None
                if u.fillers:
                    fn, filler_open = u.fillers.pop(0)
                    fn()
                if len(ctxq) >= LAG:
                    drain_one()
                u.scores(jt)
                ctxq.append((u, jt))
            for fn, _ in u.fillers:
                fn()
            filler_open = False
            u.fillers = []
        while ctxq:
            drain_one()
        if late_epi is not None:
            late_epi.epilogue_late()
            late_epi = None
        # tail: the last i-block's output projection.  The scores/pctx
        # pools are idle now, so each s-tile gets its own PSUM space
        # (po1+po2 packed into one [128,2,512] scores-shaped tile or a
        # pctx tile pair) — no pool WAR serialization across the four
        # tiles, they pipeline at PE/DVE/DMA throughput.
        def emit_outproj_tail(st, po1, po2, dmaq):
            for m in range(MT):
                nc.tensor.matmul(
                    po1, lhsT=ctxt[:, m, st * P:(st + 1) * P],
                    rhs=wo[:, m, 0:IB], start=(m == 0), stop=(m == MT - 1))
            for m in range(MT):
                nc.tensor.matmul(
                    po2, lhsT=ctxt[:, m, st * P:(st + 1) * P],
                    rhs=wo[:, m, IB:D], start=(m == 0), stop=(m == MT - 1))
            # split the cast so the first half's DMA starts while the
            # second half converts; the four tiles' stores go out on
            # different DMA queues and drain in parallel
            osb = ob.tile([P, D], CDT, tag="osb")
            nc.vector.tensor_copy(osb[:, 0:IB], po1)
            dmaq.dma_start(out=out_d[st * P:(st + 1) * P, 0:IB],
                           in_=osb[:, 0:IB])
            nc.vector.tensor_copy(osb[:, IB:D], po2)
            dmaq.dma_start(out=out_d[st * P:(st + 1) * P, IB:D],
                           in_=osb[:, IB:D])

        tp1 = pp.tile([P, 2, IB], F32, tag="ps", name="tail12")
        emit_outproj_tail(12, tp1[:, 0, :], tp1[:, 1, 0:D - IB], nc.sync)
        tp2 = pp.tile([P, 2, IB], F32, tag="ps", name="tail13")
        emit_outproj_tail(13, tp2[:, 0, :], tp2[:, 1, 0:D - IB], nc.scalar)
        tc1 = cp.tile([P, IB], F32, tag="pc", name="tail14a")
        tc2 = cp.tile([P, IB], F32, tag="pc", name="tail14b")
        emit_outproj_tail(14, tc1, tc2[:, 0:D - IB], nc.gpsimd)
        to1 = op.tile([P, IB], F32, tag="po", name="tail15a")
        to2 = op.tile([P, D - IB], F32, tag="po", name="tail15b")
        emit_outproj_tail(15, to1, to2, nc.sync)


def build_nc():
    nc = bacc.Bacc(trn_type="TRN2", target_bir_lowering=False, debug=False)
    xt_d = nc.dram_tensor("xt", [D, S], CDT, kind="ExternalInput").ap()
    wq_d = nc.dram_tensor("wq", [D, DL], CDT, kind="ExternalInput").ap()
    wk_d = nc.dram_tensor("wk", [D, DL], CDT, kind="ExternalInput").ap()
    wv_d = nc.dram_tensor("wv", [D, DL], CDT, kind="ExternalInput").ap()
    wo_d = nc.dram_tensor("wo", [DL, D], CDT, kind="ExternalInput").ap()
    out_d = nc.dram_tensor("out", [S, D], CDT, kind="ExternalOutput").ap()
    with tile.TileContext(nc) as tc:
        _emit(nc, tc, xt_d, wq_d, wk_d, wv_d, wo_d, out_d)
    nc.compile()
    return nc


def make_in_maps(x, Wq, Wk, Wv, Wo):
    in_maps = []
    for c in range(N_CORES):
        b, g = c // 2, c % 2
        hsl = slice(g * DL, (g + 1) * DL)
        in_maps.append({
            "xt": np.ascontiguousarray(x[b].T).astype(NP_CDT),
            "wq": np.ascontiguousarray(Wq[hsl, :].T).astype(NP_CDT),
            "wk": np.ascontiguousarray(Wk[hsl, :].T).astype(NP_CDT),
            "wv": np.ascontiguousarray(Wv[hsl, :].T).astype(NP_CDT),
            "wo": np.ascontiguousarray(Wo[:, hsl].T).astype(NP_CDT),
        })
    return in_maps


_BUILT = None
LAST_RESULT = None


def _install_ntff_hook():
    """Provide the antenv.axon_hooks module run_bass_kernel_spmd expects
    for NTFF profiling under axon (the agent image ships only a stub
    antenv package)."""
    import sys
    import types
    if "antenv.axon_hooks" in sys.modules:
        return
    mod = types.ModuleType("antenv.axon_hooks")
    mod._hook = None

    def set_axon_ntff_profile_hook(h):
        mod._hook = h

    def get_axon_ntff_profile_hook():
        return mod._hook

    mod.set_axon_ntff_profile_hook = set_axon_ntff_profile_hook
    mod.get_axon_ntff_profile_hook = get_axon_ntff_profile_hook
    sys.modules["antenv.axon_hooks"] = mod
    import antenv
    antenv.axon_hooks = mod
    try:
        from trn_agent_boot.trn_boot import _ntff_profile_via_ctypes
        hook = _ntff_profile_via_ctypes("/opt/axon/libaxon_pjrt.so")
        if hook is not None:
            mod._hook = hook
    except Exception:
        pass


def kernel(**inputs):
    global _BUILT, LAST_RESULT
    from concourse.bass_utils import run_bass_kernel_spmd

    x = np.asarray(inputs["x"], np.float32)
    Wq = np.asarray(inputs["Wq"], np.float32)
    Wk = np.asarray(inputs["Wk"], np.float32)
    Wv = np.asarray(inputs["Wv"], np.float32)
    Wo = np.asarray(inputs["Wo"], np.float32)
    bo = np.asarray(inputs["bo"], np.float32)

    if _BUILT is None:
        _BUILT = build_nc()
    nc = _BUILT

    trace = bool(int(os.environ.get("KTRACE", "0")))
    if trace:
        _install_ntff_hook()
    in_maps = make_in_maps(x, Wq, Wk, Wv, Wo)
    res = run_bass_kernel_spmd(
        nc, in_maps, core_ids=list(range(N_CORES)), trace=trace)
    LAST_RESULT = res

    out = np.empty((B, S, D), np.float32)
    for b in range(B):
        out[b] = np.asarray(res.results[2 * b]["out"], np.float32)
        out[b] += np.asarray(res.results[2 * b + 1]["out"], np.float32)
    out += bo
    return out
